# revision 8
# baseline (speedup 1.0000x reference)
"""CoAttention kernel for 8 TRN2 NeuronCores (Bass/Tile, SPMD).

Problem: B=4 batches x 2 attention directions = 8 independent co-attention
computations -> one per core.  Per core (batch b, direction d):
    Q = wq @ qf + bq        [256, 2304]     (qf = query-side features)
    K = wr @ rf + br        [256, 2304]     (rf = reference-side features)
    S^T = K^T Q             [2304, 2304]    (computed in m-strips of 128)
    attnT = exp(S^T - 40)   (bf16, unnormalized; softmax denom applied at end)
    sums[q] = sum_m attnT[m, q]   (DVE strip-sum chain + one f32 ones-matmul)
    out = (rf @ attnT) * (1/sums)           [2048, 2304]  (bf16 out)
Host assembles: left_att = concat(left, out[b,dir=0]), right_att likewise.

Precision: inputs/weights and Q/K in fp16 (halves the phase-1 HBM traffic
that made the projection phase DMA-bound; PE accumulates in fp32 so score
noise is dominated by the same accumulator rounding as the f32r path),
attn@V in bf16, output stored bf16 (upcast on host).  No row-max
subtraction: scores are |S| <~ 80, exp(S-40) stays in fp32/bf16 range;
normalization is exact math.

Walrus in this toolchain allows ONE sync-wait per instruction; SafeTileContext
splits multi-wait instructions into standalone wait ops, and splits the
end-of-kernel drain the same way.
"""
import numpy as np
import ml_dtypes

import concourse.bass as bass
import concourse.mybir as mybir
import concourse.tile as tile
from concourse.vector_clock import ScopedClock
from concourse.bass_utils import run_bass_kernel_spmd

B = 4
C = 2048
HW = 48 * 48          # 2304
D = 256
NCORES = 8

CB = C // 128         # 16 c-blocks
DB = D // 128         # 2 d-blocks
MS = HW // 128        # 18 m-strips
# phase-1 n chunks: 512-wide + 256 tail; each chunk's psum fits one 2KB bank
P1CHUNKS = [(0, 512), (512, 512), (1024, 512), (1536, 512), (2048, 256)]
NQT = 3               # phase-2 q thirds
QT = HW // NQT        # 768
# sub-chunks within a q-third: matmul outputs must not cross a 2KB PSUM
# bank boundary, so split 768 = 512 + 256
SUBS = [(0, 512), (512, 256)]

F32 = mybir.dt.float32
F16 = mybir.dt.float16
BF16 = mybir.dt.bfloat16

# module-level knobs / results (used by test.py)
TRACE = False
LAST_RESULT = None


class SafeTileContext(tile.TileContext):
    """This walrus build allows at most ONE sync wait per instruction.
    Hoist extra waits onto standalone EventSemaphore (wait-only) ops placed
    immediately before, on the same engine queue; same for the final drain."""
    MAX_WAITS = 1

    def _lower_ordered_insts(self, ordered):
        for bname, insts in ordered.items():
            new_list = []
            for inst in insts:
                si = inst.sync_info
                if si is not None and len(si.on_wait) > self.MAX_WAITS:
                    waits = list(si.on_wait)
                    movable = [w for w in waits if w.wait_reg is None]
                    fixed = [w for w in waits if w.wait_reg is not None]
                    keep = fixed + movable[-1:] if movable else fixed
                    hoist = movable[:-1] if movable else []
                    for w in hoist:
                        wi = mybir.InstEventSemaphore(
                            name=self.nc.get_next_instruction_name(),
                            ins=[], outs=[])
                        wi.engine = inst.engine
                        wi.sync_info = mybir.SyncInfo(on_wait=[w], on_update=[])
                        new_list.append(wi)
                    inst.sync_info = mybir.SyncInfo(
                        on_wait=keep, on_update=list(si.on_update))
                new_list.append(inst)
            insts[:] = new_list
        super()._lower_ordered_insts(ordered)

    def _drain_and_barrier(self, tick_clock, wait_clock):
        drain_inst = self.nc.sync.drain()
        wait_clock.add_sem_waits(
            drain_inst.ins, ScopedClock({None: tick_clock.global_clock}))
        si = drain_inst.ins.sync_info
        waits = list(si.on_wait) if si is not None else []
        ups = list(si.on_update) if si is not None else []
        if len(waits) > self.MAX_WAITS:
            drain_inst.ins.sync_info = mybir.SyncInfo(
                on_wait=waits[: self.MAX_WAITS], on_update=ups)
            rest = waits[self.MAX_WAITS:]
            for i in range(0, len(rest), self.MAX_WAITS):
                extra = self.nc.sync.drain()
                extra.ins.sync_info = mybir.SyncInfo(
                    on_wait=rest[i : i + self.MAX_WAITS], on_update=[])
        self.nc.all_engine_barrier()
        assert self.sems is not None
        popped = self.nc._tile_sem_poison_stack.pop()
        assert popped is self._sem_poison
        self.nc.clear_and_free_semaphores(list(self.sems.allocated().values()))
        self.nc.all_engine_barrier()


def build_kernel():
    nc = bass.Bass("TRN2", target_bir_lowering=False, debug=False)

    qf = nc.dram_tensor("qf", [C, HW], F16, kind="ExternalInput")
    rf = nc.dram_tensor("rf", [C, HW], F16, kind="ExternalInput")
    vtb = nc.dram_tensor("vtb", [HW, C], BF16, kind="ExternalInput")
    wqt = nc.dram_tensor("wqt", [C, D], F16, kind="ExternalInput")
    wrt = nc.dram_tensor("wrt", [C, D], F16, kind="ExternalInput")
    bq = nc.dram_tensor("bq", [128, DB], F32, kind="ExternalInput")
    br = nc.dram_tensor("br", [128, DB], F32, kind="ExternalInput")
    out = nc.dram_tensor("out", [C, HW], BF16, kind="ExternalOutput")

    with SafeTileContext(nc) as tc:
        with tc.tile_pool(name="persist", bufs=1) as persist, \
             tc.tile_pool(name="dsc", bufs=1, space="DRAM") as dram_scratch:
            # ---- persistent tiles ----
            q_sb = persist.tile([128, DB, HW], F16)     # Q  [d, n]
            k_sb = persist.tile([128, DB, HW], F16)     # K  [d, n]
            vt = persist.tile([128, MS, C], BF16)       # V^T [m, c]
            bq_t = persist.tile([128, DB], F32)
            br_t = persist.tile([128, DB], F32)
            nbias = persist.tile([128, 1], F32)
            nc.vector.memset(nbias, -40.0)
            ones = persist.tile([128, 1], F32)
            nc.vector.memset(ones, 1.0)

            # ================= phase 1: projections + V^T =================
            with tc.tile_pool(name="wpool", bufs=1) as wpool, \
                 tc.tile_pool(name="xstream", bufs=4) as xstream, \
                 tc.tile_pool(name="p1ps", bufs=1, space="PSUM") as p1ps:
                wq_sb = wpool.tile([128, CB, D], F16)
                wr_sb = wpool.tile([128, CB, D], F16)
                wqr = wqt.ap().rearrange("(k p) d -> p k d", p=128)
                wrr = wrt.ap().rearrange("(k p) d -> p k d", p=128)
                # issue order matters: the sync engine needs ~0.65us per
                # dma_start, so interleave the weight halves with the first
                # x-stream loads to get the first matmul going ASAP
                lo, hi = slice(0, 8), slice(8, 16)
                nc.sync.dma_start(out=wq_sb[:, lo, :], in_=wqr[:, lo, :])
                nc.sync.dma_start(out=wr_sb[:, lo, :], in_=wrr[:, lo, :])

                for ch, (coff, csz) in enumerate(P1CHUNKS):
                    qps = [p1ps.tile([128, 512], F32, tag=f"qps{d}",
                                     name=f"qps{d}_{ch}")
                           for d in range(DB)]
                    kps = [p1ps.tile([128, 512], F32, tag=f"kps{d}",
                                     name=f"kps{d}_{ch}")
                           for d in range(DB)]
                    qfr = qf.ap().rearrange("(k p) n -> p k n", p=128)
                    rfr = rf.ap().rearrange("(k p) n -> p k n", p=128)
                    for quad in range(CB // 4):
                        xq = xstream.tile([128, 4, 512], F16, tag="xq",
                                          name=f"xq_{ch}_{quad}")
                        nc.sync.dma_start(
                            out=xq[:, :, :csz],
                            in_=qfr[:, quad * 4:(quad + 1) * 4,
                                    coff:coff + csz])
                        xr = xstream.tile([128, 4, 512], F16, tag="xr",
                                          name=f"xr_{ch}_{quad}")
                        nc.sync.dma_start(
                            out=xr[:, :, :csz],
                            in_=rfr[:, quad * 4:(quad + 1) * 4,
                                    coff:coff + csz])
                        if ch == 0 and quad == 0:
                            nc.sync.dma_start(out=wq_sb[:, hi, :],
                                              in_=wqr[:, hi, :])
                            nc.sync.dma_start(out=wr_sb[:, hi, :],
                                              in_=wrr[:, hi, :])
                            nc.sync.dma_start(out=bq_t, in_=bq.ap())
                            nc.sync.dma_start(out=br_t, in_=br.ap())
                        for i in range(4):
                            c = quad * 4 + i
                            for d in range(DB):
                                nc.tensor.matmul(
                                    qps[d][:, :csz],
                                    wq_sb[:, c, d * 128:(d + 1) * 128],
                                    xq[:, i, :csz],
                                    start=(c == 0), stop=(c == CB - 1))
                                nc.tensor.matmul(
                                    kps[d][:, :csz],
                                    wr_sb[:, c, d * 128:(d + 1) * 128],
                                    xr[:, i, :csz],
                                    start=(c == 0), stop=(c == CB - 1))
                    for d in range(DB):
                        nc.vector.tensor_scalar_add(
                            q_sb[:, d, coff:coff + csz],
                            qps[d][:, :csz], bq_t[:, d:d + 1])
                        nc.vector.tensor_scalar_add(
                            k_sb[:, d, coff:coff + csz],
                            kps[d][:, :csz], br_t[:, d:d + 1])

            # V^T (host-pretransposed bf16), loaded in c-quarters: av's
            # cb loop consumes vt in c order across ALL strips, so c-major
            # chunks let av(0) start after the first quarter instead of
            # waiting for the whole 9.4MB load
            vtr = vtb.ap().rearrange("(s p) c -> p s c", p=128)
            for h in range(4):
                cs = slice(512 * h, 512 * (h + 1))
                nc.sync.dma_start(out=vt[:, :, cs], in_=vtr[:, :, cs])

            # ================= phase 2: S^T, softmax, attn@V ==============
            with tc.tile_pool(name="attn", bufs=2) as attnp, \
                 tc.tile_pool(name="small", bufs=2) as small, \
                 tc.tile_pool(name="ostage", bufs=3) as ostage, \
                 tc.tile_pool(name="sps", bufs=2, space="PSUM") as spsp, \
                 tc.tile_pool(name="zps", bufs=1, space="PSUM") as zpsp, \
                 tc.tile_pool(name="ops", bufs=2, space="PSUM") as opsp:
                attn_ts = {}
                bcs = {}

                accs = {}

                def st_phase(qt):
                    attn_t = attnp.tile([128, MS, QT], BF16, tag="attnT",
                                        name=f"attnT_{qt}")
                    attn_ts[qt] = attn_t
                    # f32 running strip-sum on GpSimd (replaces per-strip
                    # ones-matmuls; DVE is busy with the av evacuations)
                    acc = small.tile([128, QT], F32, tag="acc",
                                     name=f"acc_{qt}")
                    accs[qt] = acc
                    for m in range(MS):
                        sps = spsp.tile([128, QT], F32, tag="sps")
                        for off, sz in SUBS:
                            for d in range(DB):
                                nc.tensor.matmul(
                                    sps[:, off:off + sz],
                                    k_sb[:, d, m * 128:(m + 1) * 128],
                                    q_sb[:, d, qt * QT + off:
                                         qt * QT + off + sz],
                                    start=(d == 0), stop=(d == DB - 1))
                        nc.scalar.activation(
                            attn_t[:, m, :], sps,
                            mybir.ActivationFunctionType.Exp,
                            bias=nbias, scale=1.0)
                        if m == 1:
                            nc.gpsimd.tensor_add(
                                acc, attn_t[:, 0, :], attn_t[:, 1, :])
                        elif m >= 2:
                            nc.gpsimd.tensor_add(acc, acc, attn_t[:, m, :])

                def zfin(qt):
                    # 128-partition reduction of acc via one f32 ones-matmul,
                    # then 1/Z broadcast.  Emitted a few cb into the PREVIOUS
                    # av phase so the PE never waits on the strip-sum chain.
                    acc = accs.pop(qt)
                    z_ps = zpsp.tile([1, QT], F32, tag="zps",
                                     name=f"zps_{qt}")
                    for off, sz in SUBS:
                        nc.tensor.matmul(z_ps[:, off:off + sz], ones,
                                         acc[:, off:off + sz],
                                         start=True, stop=True)
                    sums_sb = small.tile([1, QT], F32, tag="sums_sb",
                                         name=f"sums_sb_{qt}")
                    for off, sz in SUBS:
                        nc.scalar.copy(
                            sums_sb[:, off:off + sz], z_ps[:, off:off + sz])
                    invs = small.tile([1, QT], F32, tag="invs",
                                      name=f"invs_{qt}")
                    nc.vector.reciprocal(invs, sums_sb)
                    invs_dram = dram_scratch.tile([1, QT], F32, tag="invd",
                                                  name=f"invd_{qt}", bufs=2)
                    nc.sync.dma_start(out=invs_dram, in_=invs)
                    bc = small.tile([128, QT], F32, tag="bc", name=f"bc_{qt}")
                    bcs[qt] = bc
                    nc.sync.dma_start(out=bc, in_=invs_dram.partition_broadcast(128))

                def av_phase(qt, inject=None):
                    attn_t = attn_ts.pop(qt)
                    bc = bcs.pop(qt)
                    for cb in range(CB):
                        if cb == 4 and inject is not None:
                            inject()
                        o_sb = ostage.tile([128, QT], BF16, tag="osb",
                                           name=f"osb_{qt}_{cb}")
                        for off, sz in SUBS:
                            ops = opsp.tile([128, 512], F32, tag="ops")
                            nc.tensor.matmul(
                                ops[:, :sz], vt[:, 0, cb * 128:(cb + 1) * 128],
                                attn_t[:, 0, off:off + sz],
                                start=True, stop=False)
                            for m in range(1, MS):
                                nc.tensor.matmul(
                                    ops[:, :sz], vt[:, m, cb * 128:(cb + 1) * 128],
                                    attn_t[:, m, off:off + sz],
                                    start=False, stop=(m == MS - 1))
                            nc.vector.scalar_tensor_tensor(
                                o_sb[:, off:off + sz], ops[:, :sz], 0.0,
                                bc[:, off:off + sz],
                                op0=mybir.AluOpType.add,
                                op1=mybir.AluOpType.mult)
                        nc.gpsimd.dma_start(
                            out=out.ap()[cb * 128:(cb + 1) * 128,
                                         qt * QT:(qt + 1) * QT],
                            in_=o_sb)

                st_phase(0)
                st_phase(1)
                zfin(0)
                av_phase(0, inject=lambda: zfin(1))
                st_phase(2)
                av_phase(1, inject=lambda: zfin(2))
                av_phase(2)
    return nc


def kernel(left_features, right_features, wq, bq, wr, br):
    global LAST_RESULT
    left = np.asarray(left_features, dtype=np.float32)
    right = np.asarray(right_features, dtype=np.float32)
    wq = np.asarray(wq, dtype=np.float32)
    wr = np.asarray(wr, dtype=np.float32)
    bq = np.asarray(bq, dtype=np.float32)
    br = np.asarray(br, dtype=np.float32)

    lf = left.reshape(B, C, HW).astype(np.float16)
    rg = right.reshape(B, C, HW).astype(np.float16)
    wqt = np.ascontiguousarray(wq.T).astype(np.float16)   # [C, D]
    wrt = np.ascontiguousarray(wr.T).astype(np.float16)
    bq_t = np.ascontiguousarray(bq.reshape(DB, 128).T)    # [128, DB]
    br_t = np.ascontiguousarray(br.reshape(DB, 128).T)

    nc = build_kernel()
    in_maps = []
    for core in range(NCORES):
        b, d = core // 2, core % 2
        qf_c = lf[b] if d == 0 else rg[b]
        rf_c = rg[b] if d == 0 else lf[b]
        in_maps.append({
            "qf": np.ascontiguousarray(qf_c),
            "rf": np.ascontiguousarray(rf_c),
            "vtb": np.ascontiguousarray(
                rf_c.T.astype(np.float32).astype(ml_dtypes.bfloat16)),
            "wqt": wqt, "wrt": wrt, "bq": bq_t, "br": br_t,
        })
    res = run_bass_kernel_spmd(nc, in_maps, core_ids=list(range(NCORES)),
                               trace=TRACE)
    LAST_RESULT = res

    weighted = np.stack([res.results[core]["out"].astype(np.float32)
                         for core in range(NCORES)])
    weighted = weighted.reshape(B, 2, C, 48, 48)
    left_att = np.concatenate([left, weighted[:, 0]], axis=1)
    right_att = np.concatenate([right, weighted[:, 1]], axis=1)
    return (left_att, right_att)


# revision 10
# speedup vs baseline: 1.1037x; 1.1037x over previous
"""CoAttention kernel for 8 TRN2 NeuronCores (Bass/Tile, SPMD).

Problem: B=4 batches x 2 attention directions = 8 independent co-attention
computations -> one per core.  Per core (batch b, direction d):
    Q = wq @ qf + bq        [256, 2304]     (qf = query-side features)
    K = wr @ rf + br        [256, 2304]     (rf = reference-side features)
    S^T = K^T Q             [2304, 2304]    (computed in m-strips of 128)
    attnT = exp(S^T - 40)   (bf16, unnormalized; softmax denom applied at end)
    sums[q] = sum_m attnT[m, q]   (DVE strip-sum chain + one f32 ones-matmul)
    out = (rf @ attnT) * (1/sums)           [2048, 2304]  (bf16 out)
Host assembles: left_att = concat(left, out[b,dir=0]), right_att likewise.

Precision: inputs/weights and Q/K in fp16 (halves the phase-1 HBM traffic
that made the projection phase DMA-bound; PE accumulates in fp32 so score
noise is dominated by the same accumulator rounding as the f32r path),
attn@V in bf16, output stored bf16 (upcast on host).  No row-max
subtraction: scores are |S| <~ 80, exp(S-40) stays in fp32/bf16 range;
normalization is exact math.

Walrus in this toolchain allows ONE sync-wait per instruction; SafeTileContext
splits multi-wait instructions into standalone wait ops, and splits the
end-of-kernel drain the same way.
"""
import numpy as np
import ml_dtypes

import concourse.bass as bass
import concourse.mybir as mybir
import concourse.tile as tile
from concourse.vector_clock import ScopedClock
from concourse.bass_utils import run_bass_kernel_spmd

B = 4
C = 2048
HW = 48 * 48          # 2304
D = 256
NCORES = 8

CB = C // 128         # 16 c-blocks
DB = D // 128         # 2 d-blocks
MS = HW // 128        # 18 m-strips
# phase-1 n chunks: 512-wide + 256 tail; each chunk's psum fits one 2KB bank
P1CHUNKS = [(0, 512), (512, 512), (1024, 512), (1536, 512), (2048, 256)]
NQT = 3               # phase-2 q thirds
QT = HW // NQT        # 768
# sub-chunks within a q-third: matmul outputs must not cross a 2KB PSUM
# bank boundary, so split 768 = 512 + 256
SUBS = [(0, 512), (512, 256)]

F32 = mybir.dt.float32
F16 = mybir.dt.float16
BF16 = mybir.dt.bfloat16

# module-level knobs / results (used by test.py)
TRACE = False
LAST_RESULT = None


class SafeTileContext(tile.TileContext):
    """This walrus build allows at most ONE sync wait per instruction.
    Hoist extra waits onto standalone EventSemaphore (wait-only) ops placed
    immediately before, on the same engine queue; same for the final drain."""
    MAX_WAITS = 1

    def _lower_ordered_insts(self, ordered):
        for bname, insts in ordered.items():
            new_list = []
            for inst in insts:
                si = inst.sync_info
                if si is not None and len(si.on_wait) > self.MAX_WAITS:
                    waits = list(si.on_wait)
                    movable = [w for w in waits if w.wait_reg is None]
                    fixed = [w for w in waits if w.wait_reg is not None]
                    keep = fixed + movable[-1:] if movable else fixed
                    hoist = movable[:-1] if movable else []
                    for w in hoist:
                        wi = mybir.InstEventSemaphore(
                            name=self.nc.get_next_instruction_name(),
                            ins=[], outs=[])
                        wi.engine = inst.engine
                        wi.sync_info = mybir.SyncInfo(on_wait=[w], on_update=[])
                        new_list.append(wi)
                    inst.sync_info = mybir.SyncInfo(
                        on_wait=keep, on_update=list(si.on_update))
                new_list.append(inst)
            insts[:] = new_list
        super()._lower_ordered_insts(ordered)

    def _drain_and_barrier(self, tick_clock, wait_clock):
        drain_inst = self.nc.sync.drain()
        wait_clock.add_sem_waits(
            drain_inst.ins, ScopedClock({None: tick_clock.global_clock}))
        si = drain_inst.ins.sync_info
        waits = list(si.on_wait) if si is not None else []
        ups = list(si.on_update) if si is not None else []
        if len(waits) > self.MAX_WAITS:
            drain_inst.ins.sync_info = mybir.SyncInfo(
                on_wait=waits[: self.MAX_WAITS], on_update=ups)
            rest = waits[self.MAX_WAITS:]
            for i in range(0, len(rest), self.MAX_WAITS):
                extra = self.nc.sync.drain()
                extra.ins.sync_info = mybir.SyncInfo(
                    on_wait=rest[i : i + self.MAX_WAITS], on_update=[])
        self.nc.all_engine_barrier()
        assert self.sems is not None
        popped = self.nc._tile_sem_poison_stack.pop()
        assert popped is self._sem_poison
        self.nc.clear_and_free_semaphores(list(self.sems.allocated().values()))
        self.nc.all_engine_barrier()


def build_kernel():
    nc = bass.Bass("TRN2", target_bir_lowering=False, debug=False)

    qf = nc.dram_tensor("qf", [C, HW], F16, kind="ExternalInput")
    rf = nc.dram_tensor("rf", [C, HW], F16, kind="ExternalInput")
    vtb = nc.dram_tensor("vtb", [HW, C], BF16, kind="ExternalInput")
    wqt = nc.dram_tensor("wqt", [C, D], F16, kind="ExternalInput")
    wrt = nc.dram_tensor("wrt", [C, D], F16, kind="ExternalInput")
    bq = nc.dram_tensor("bq", [128, DB], F32, kind="ExternalInput")
    br = nc.dram_tensor("br", [128, DB], F32, kind="ExternalInput")
    out = nc.dram_tensor("out", [C, HW], BF16, kind="ExternalOutput")

    with SafeTileContext(nc) as tc:
        with tc.tile_pool(name="persist", bufs=1) as persist, \
             tc.tile_pool(name="dsc", bufs=1, space="DRAM") as dram_scratch:
            # ---- persistent tiles ----
            q_sb = persist.tile([128, DB, HW], F16)     # Q  [d, n]
            k_sb = persist.tile([128, DB, HW], F16)     # K  [d, n]
            vt = persist.tile([128, MS, C], BF16)       # V^T [m, c]
            bq_t = persist.tile([128, DB], F32)
            br_t = persist.tile([128, DB], F32)
            nbias = persist.tile([128, 1], F32)
            nc.vector.memset(nbias, -40.0)
            ones = persist.tile([128, 1], F32)
            nc.vector.memset(ones, 1.0)

            # ================= phase 1: projections + V^T =================
            with tc.tile_pool(name="wpool", bufs=1) as wpool, \
                 tc.tile_pool(name="xstream", bufs=4) as xstream, \
                 tc.tile_pool(name="p1ps", bufs=1, space="PSUM") as p1ps:
                wq_sb = wpool.tile([128, CB, D], F16)
                wr_sb = wpool.tile([128, CB, D], F16)
                wqr = wqt.ap().rearrange("(k p) d -> p k d", p=128)
                wrr = wrt.ap().rearrange("(k p) d -> p k d", p=128)
                # issue order matters: the sync engine needs ~0.65us per
                # dma_start, so interleave the weight halves with the first
                # x-stream loads to get the first matmul going ASAP
                lo, hi = slice(0, 8), slice(8, 16)
                nc.sync.dma_start(out=wq_sb[:, lo, :], in_=wqr[:, lo, :])
                nc.sync.dma_start(out=wr_sb[:, lo, :], in_=wrr[:, lo, :])

                for ch, (coff, csz) in enumerate(P1CHUNKS):
                    qps = [p1ps.tile([128, 512], F32, tag=f"qps{d}",
                                     name=f"qps{d}_{ch}")
                           for d in range(DB)]
                    kps = [p1ps.tile([128, 512], F32, tag=f"kps{d}",
                                     name=f"kps{d}_{ch}")
                           for d in range(DB)]
                    qfr = qf.ap().rearrange("(k p) n -> p k n", p=128)
                    rfr = rf.ap().rearrange("(k p) n -> p k n", p=128)
                    for quad in range(CB // 4):
                        xq = xstream.tile([128, 4, 512], F16, tag="xq",
                                          name=f"xq_{ch}_{quad}")
                        nc.sync.dma_start(
                            out=xq[:, :, :csz],
                            in_=qfr[:, quad * 4:(quad + 1) * 4,
                                    coff:coff + csz])
                        xr = xstream.tile([128, 4, 512], F16, tag="xr",
                                          name=f"xr_{ch}_{quad}")
                        nc.sync.dma_start(
                            out=xr[:, :, :csz],
                            in_=rfr[:, quad * 4:(quad + 1) * 4,
                                    coff:coff + csz])
                        if ch == 0 and quad == 1:
                            nc.sync.dma_start(out=wq_sb[:, hi, :],
                                              in_=wqr[:, hi, :])
                            nc.sync.dma_start(out=wr_sb[:, hi, :],
                                              in_=wrr[:, hi, :])
                        if ch == 0 and quad == 2:
                            nc.sync.dma_start(out=bq_t, in_=bq.ap())
                            nc.sync.dma_start(out=br_t, in_=br.ap())
                        for i in range(4):
                            c = quad * 4 + i
                            for d in range(DB):
                                nc.tensor.matmul(
                                    qps[d][:, :csz],
                                    wq_sb[:, c, d * 128:(d + 1) * 128],
                                    xq[:, i, :csz],
                                    start=(c == 0), stop=(c == CB - 1))
                                nc.tensor.matmul(
                                    kps[d][:, :csz],
                                    wr_sb[:, c, d * 128:(d + 1) * 128],
                                    xr[:, i, :csz],
                                    start=(c == 0), stop=(c == CB - 1))
                    for d in range(DB):
                        nc.vector.tensor_scalar_add(
                            q_sb[:, d, coff:coff + csz],
                            qps[d][:, :csz], bq_t[:, d:d + 1])
                        nc.vector.tensor_scalar_add(
                            k_sb[:, d, coff:coff + csz],
                            kps[d][:, :csz], br_t[:, d:d + 1])

            # V^T (host-pretransposed bf16), loaded in c-quarters: av's
            # cb loop consumes vt in c order across ALL strips, so c-major
            # chunks let av(0) start after the first quarter instead of
            # waiting for the whole 9.4MB load
            vtr = vtb.ap().rearrange("(s p) c -> p s c", p=128)
            for h in range(4):
                cs = slice(512 * h, 512 * (h + 1))
                nc.sync.dma_start(out=vt[:, :, cs], in_=vtr[:, :, cs])

            # ================= phase 2: S^T, softmax, attn@V ==============
            with tc.tile_pool(name="attn", bufs=2) as attnp, \
                 tc.tile_pool(name="small", bufs=2) as small, \
                 tc.tile_pool(name="ostage", bufs=3) as ostage, \
                 tc.tile_pool(name="sps", bufs=2, space="PSUM") as spsp, \
                 tc.tile_pool(name="zps", bufs=1, space="PSUM") as zpsp, \
                 tc.tile_pool(name="ops", bufs=2, space="PSUM") as opsp:
                attn_ts = {}
                bcs = {}

                accs = {}

                def st_phase(qt):
                    attn_t = attnp.tile([128, MS, QT], BF16, tag="attnT",
                                        name=f"attnT_{qt}")
                    attn_ts[qt] = attn_t
                    # f32 running strip-sum on DVE (replaces per-strip
                    # ones-matmuls; the chain trails the exps by a few us,
                    # which the deferred zfin absorbs)
                    acc = small.tile([128, QT], F32, tag="acc",
                                     name=f"acc_{qt}")
                    accs[qt] = acc
                    for m in range(MS):
                        sps = spsp.tile([128, QT], F32, tag="sps")
                        for off, sz in SUBS:
                            for d in range(DB):
                                nc.tensor.matmul(
                                    sps[:, off:off + sz],
                                    k_sb[:, d, m * 128:(m + 1) * 128],
                                    q_sb[:, d, qt * QT + off:
                                         qt * QT + off + sz],
                                    start=(d == 0), stop=(d == DB - 1))
                        nc.scalar.activation(
                            attn_t[:, m, :], sps,
                            mybir.ActivationFunctionType.Exp,
                            bias=nbias, scale=1.0)
                        if m == 1:
                            nc.vector.tensor_add(
                                acc, attn_t[:, 0, :], attn_t[:, 1, :])
                        elif m >= 2:
                            nc.vector.tensor_add(acc, acc, attn_t[:, m, :])

                def zfin(qt):
                    # 128-partition reduction of acc via one f32 ones-matmul,
                    # then 1/Z broadcast.  Emitted a few cb into the PREVIOUS
                    # av phase so the PE never waits on the strip-sum chain.
                    acc = accs.pop(qt)
                    z_ps = zpsp.tile([1, QT], F32, tag="zps",
                                     name=f"zps_{qt}")
                    for off, sz in SUBS:
                        nc.tensor.matmul(z_ps[:, off:off + sz], ones,
                                         acc[:, off:off + sz],
                                         start=True, stop=True)
                    sums_sb = small.tile([1, QT], F32, tag="sums_sb",
                                         name=f"sums_sb_{qt}")
                    for off, sz in SUBS:
                        nc.scalar.copy(
                            sums_sb[:, off:off + sz], z_ps[:, off:off + sz])
                    invs = small.tile([1, QT], F32, tag="invs",
                                      name=f"invs_{qt}")
                    nc.vector.reciprocal(invs, sums_sb)
                    invs_dram = dram_scratch.tile([1, QT], F32, tag="invd",
                                                  name=f"invd_{qt}", bufs=2)
                    nc.sync.dma_start(out=invs_dram, in_=invs)
                    bc = small.tile([128, QT], F32, tag="bc", name=f"bc_{qt}")
                    bcs[qt] = bc
                    nc.sync.dma_start(out=bc, in_=invs_dram.partition_broadcast(128))

                def av_phase(qt, inject=None):
                    attn_t = attn_ts.pop(qt)
                    bc = bcs.pop(qt)
                    for cb in range(CB):
                        if cb == 4 and inject is not None:
                            inject()
                        o_sb = ostage.tile([128, QT], BF16, tag="osb",
                                           name=f"osb_{qt}_{cb}")
                        for off, sz in SUBS:
                            ops = opsp.tile([128, 512], F32, tag="ops")
                            nc.tensor.matmul(
                                ops[:, :sz], vt[:, 0, cb * 128:(cb + 1) * 128],
                                attn_t[:, 0, off:off + sz],
                                start=True, stop=False)
                            for m in range(1, MS):
                                nc.tensor.matmul(
                                    ops[:, :sz], vt[:, m, cb * 128:(cb + 1) * 128],
                                    attn_t[:, m, off:off + sz],
                                    start=False, stop=(m == MS - 1))
                            nc.vector.scalar_tensor_tensor(
                                o_sb[:, off:off + sz], ops[:, :sz], 0.0,
                                bc[:, off:off + sz],
                                op0=mybir.AluOpType.add,
                                op1=mybir.AluOpType.mult)
                        nc.gpsimd.dma_start(
                            out=out.ap()[cb * 128:(cb + 1) * 128,
                                         qt * QT:(qt + 1) * QT],
                            in_=o_sb)

                st_phase(0)
                st_phase(1)
                zfin(0)
                av_phase(0, inject=lambda: zfin(1))
                st_phase(2)
                av_phase(1, inject=lambda: zfin(2))
                av_phase(2)
    return nc


def kernel(left_features, right_features, wq, bq, wr, br):
    global LAST_RESULT
    left = np.asarray(left_features, dtype=np.float32)
    right = np.asarray(right_features, dtype=np.float32)
    wq = np.asarray(wq, dtype=np.float32)
    wr = np.asarray(wr, dtype=np.float32)
    bq = np.asarray(bq, dtype=np.float32)
    br = np.asarray(br, dtype=np.float32)

    lf = left.reshape(B, C, HW).astype(np.float16)
    rg = right.reshape(B, C, HW).astype(np.float16)
    wqt = np.ascontiguousarray(wq.T).astype(np.float16)   # [C, D]
    wrt = np.ascontiguousarray(wr.T).astype(np.float16)
    bq_t = np.ascontiguousarray(bq.reshape(DB, 128).T)    # [128, DB]
    br_t = np.ascontiguousarray(br.reshape(DB, 128).T)

    nc = build_kernel()
    in_maps = []
    for core in range(NCORES):
        b, d = core // 2, core % 2
        qf_c = lf[b] if d == 0 else rg[b]
        rf_c = rg[b] if d == 0 else lf[b]
        in_maps.append({
            "qf": np.ascontiguousarray(qf_c),
            "rf": np.ascontiguousarray(rf_c),
            "vtb": np.ascontiguousarray(
                rf_c.T.astype(np.float32).astype(ml_dtypes.bfloat16)),
            "wqt": wqt, "wrt": wrt, "bq": bq_t, "br": br_t,
        })
    res = run_bass_kernel_spmd(nc, in_maps, core_ids=list(range(NCORES)),
                               trace=TRACE)
    LAST_RESULT = res

    weighted = np.stack([res.results[core]["out"].astype(np.float32)
                         for core in range(NCORES)])
    weighted = weighted.reshape(B, 2, C, 48, 48)
    left_att = np.concatenate([left, weighted[:, 0]], axis=1)
    right_att = np.concatenate([right, weighted[:, 1]], axis=1)
    return (left_att, right_att)
